# revision 1
# baseline (speedup 1.0000x reference)
"""ConcatNonLocalBlock kernel v3 for 8x Trainium2 NeuronCores.

Math: the reference's attention matrix attn[b,i,j] = s[b,i]/n is constant
along j, so the block collapses to a rank-1 correction of x:

    out[b,c,i] = xh[b,c,i] + s[b,i] * uu[b,c]
    xh      = x + bexp  (folded on host into the bf16 quantization pass)
    s[b,i]  = ReLU(wS . xh[b,:,i] + bS')    wS = Wq^T wq_c + Wk^T wk_c,
                                            bS' = bS - wS.bexp
    uu[b,:] = (Wexp Wv/N) @ xhsum[b] + (Wexp bv - Wexp Wv bexp)

Sharding: data-parallel over batch, one sample per core (B=8, 8 cores).
I/O in bf16 (rel-err budget 2e-2 >> bf16's ~1e-3): halves HBM traffic.

Schedule (single core):
  - x streams in as 4 DMA chunks; per 512-col compute chunk PE does the
    s-matvec (bf16), ACT the ReLU, DVE accumulates xsum via 4x tensor_scalar
    accum_out. PE runs warm-up dummy matmuls to hold the p-state at 2.4GHz.
  - uu is computed in both row form (for PE outer products) and column form
    (PSUM, read directly as DVE/Pool per-partition scalars).
  - Output phase per chunk, split across engines:
      DVE path : PE broadcasts s into PSUM (ones outer product), DVE STT
                 computes xh += s*uu in place.
      ACT path : PE accumulates uu (x) s + I.x into PSUM, ACT copies to SBUF.
      Pool path: gpsimd partition_broadcast of s + two gpsimd STTs.
    Out-DMA per chunk as soon as its columns are final.
  - Minimal TileContext exit (drain only): the NEFF runs once under PJRT,
    so the stock sem-clear + double barrier epilogue is dead time.
"""

import os
import sys

import numpy as np

sys.path.insert(0, "/opt/trn_rl_repo")

import concourse.bass as bass
import concourse.tile as tile
from concourse import mybir
from concourse.bass_utils import run_bass_kernel_spmd

B, C, H, W = 8, 256, 56, 56
N = H * W  # 3136
E = C // 2  # 128
P = 128
NT = 2

# compute chunks: 6 x 512 + 64
CW = 512
CCHUNKS = [(i * CW, CW) for i in range(6)] + [(6 * CW, 64)]
# DMA chunks: first one small so compute starts early
DCHUNKS = [(0, 512), (512, 1024), (1536, 1024), (2560, 576)]
# first compute chunk of each dma chunk (for PE observer placement)
DFIRST = {0: 0, 1: 1, 2: 3, 3: 5}

# output-path assignment per compute chunk: (path_t0, path_t1)
# 'D' = DVE STT (PE s-broadcast psum), 'A' = ACT copy (PE outer + I.x),
# 'P' = Pool STT (gpsimd s-broadcast sbuf)
PATHS = [
    ("D", "D"),
    ("D", "D"),
    ("A", "A"),
    ("A", "A"),
    ("A", "A"),
    ("A", "A"),
    ("D", "D"),
]
# Pool-path chunks' s-broadcast column offsets within the sb5 tile
SB5_OFF = {5: 0}

F32 = mybir.dt.float32
BF16 = mybir.dt.bfloat16

# critical smalls [128, SMC_F] f32
SMC_WS = 0  # cols 0..1 bf16 low half: wS[t*128+p]
SMC_BS = 2  # f32 [0,2] = bS'
SMC_F = 4

# big smalls [128, SMB_F] f32 (bf16 payload packed 2/f32)
SMB_WVET = 0  # cols 0..255: block t cols [t*128,(t+1)*128): B_t[k,m]=Wve[m,t*128+k]/N
SMB_WBV = 256  # cols 256..383: wexpbv' row [1,256] on partition 0
SMB_I = 384  # cols 384..447: I128 bf16
SMB_ONE = 448  # cols 448..511: ones row [1,128] bf16 on partition 0
SMB_F = 512

LAST_RESULTS = None
_prog_cache = {}


def _split_multi_waits(nc):
    """Walrus rejects >1 sync wait per instruction. Hoist extra waits onto
    engine NOPs inserted just before the offending instruction (sequencer
    dispatch is in-order, so a wait on a NOP gates everything after it)."""
    for blk in nc.m.functions[0].blocks:
        new_insts = []
        for inst in blk.instructions:
            si = getattr(inst, "sync_info", None)
            if si is not None and len(si.on_wait) > 1:
                waits = list(si.on_wait)
                for w in waits[:-1]:
                    nop = mybir.InstNoOp(
                        name=nc.get_next_instruction_name(), ins=[], outs=[]
                    )
                    nop.engine = inst.engine
                    nop.sync_info = mybir.SyncInfo(on_wait=[w], on_update=[])
                    nc.register_instruction(nop)
                    new_insts.append(nop)
                inst.sync_info = mybir.SyncInfo(
                    on_wait=[waits[-1]], on_update=list(si.on_update)
                )
            new_insts.append(inst)
        blk.instructions[:] = new_insts


class _MinimalExitTC(tile.TileContext):
    """Exit = drain only. Single-execution NEFF: skip sem clear + barriers.
    Also split multi-wait drains into single-wait NoOps (walrus constraint)."""

    def _drain_and_barrier(self, tick_clock, wait_clock):
        from concourse.vector_clock import ScopedClock

        drain_inst = self.nc.sync.drain()
        wait_clock.add_sem_waits(
            drain_inst.ins, ScopedClock({None: tick_clock.global_clock})
        )
        si = drain_inst.ins.sync_info
        if si is not None and len(si.on_wait) > 1:
            waits = list(si.on_wait)
            drain_inst.ins.sync_info = mybir.SyncInfo(
                on_wait=[], on_update=list(si.on_update)
            )
            for w in waits:
                nop = self.nc.sync.nop()
                nop.ins.sync_info = mybir.SyncInfo(on_wait=[w], on_update=[])
        popped = self.nc._tile_sem_poison_stack.pop()
        assert popped is self._sem_poison


def _build():
    nc = bass.Bass()
    xh_in = nc.dram_tensor("xh", [C, N], BF16, kind="ExternalInput")
    smc_in = nc.dram_tensor("smc", [P, SMC_F], F32, kind="ExternalInput")
    smb_in = nc.dram_tensor("smb", [P, SMB_F], F32, kind="ExternalInput")
    out = nc.dram_tensor("out", [C, N], BF16, kind="ExternalOutput")

    with _MinimalExitTC(nc) as tc:
        with (
            tc.tile_pool(name="persist", bufs=1) as persist,
            tc.tile_pool(name="ps_z", bufs=2, space="PSUM") as ps_z,
            tc.tile_pool(name="ps_u", bufs=1, space="PSUM") as ps_u,
            tc.tile_pool(name="ps_o", bufs=2, space="PSUM") as ps_o,
            tc.tile_pool(name="ps_b", bufs=1, space="PSUM") as ps_b,
        ):
            smc = persist.tile([P, SMC_F], F32, tag="smc")
            smb = persist.tile([P, SMB_F], F32, tag="smb")
            warm = persist.tile([P, CW], BF16, tag="warm")
            nc.gpsimd.memset(warm[:, :], 0.0)
            nc.gpsimd.dma_start(out=smc, in_=smc_in[:, :])
            nc.gpsimd.dma_start(out=smb, in_=smb_in[:, :])

            def smcbf(t):  # [128,1] bf16 wS column for tile t
                return smc[0:P, t : t + 1].bitcast(BF16)[:, 0:1]

            def smbbf(p0, p1, c0, c1):  # bf16 view of smb cols [c0:c1)
                return smb[p0:p1, c0:c1].bitcast(BF16)

            xh = persist.tile([P, NT, N], BF16, tag="xh")
            s_row = persist.tile([1, N], BF16, tag="s_row")
            xsp = persist.tile([P, NT, len(DCHUNKS)], F32, tag="xsp")
            xsum = persist.tile([P, NT, 1], F32, tag="xsum")
            xsum_bf = persist.tile([P, NT], BF16, tag="xsum_bf")
            uu_row = persist.tile([1, C], BF16, tag="uu_row")
            uu_col = persist.tile([P, NT], F32, tag="uu_col")
            junk = persist.tile([P, NT, len(DCHUNKS), 1024], BF16, tag="junk")
            obf = persist.tile([P, NT, N], BF16, tag="obf")
            sb5 = persist.tile([P, CW], BF16, tag="sb5")

            # one PSUM bank shared by: PE warm-up dummies (cols 0:448),
            # the row-form uu accumulation (cols 0:256, partition 0), and
            # the column-form uu (cols 448:450).
            upw = ps_u.tile([P, 450], F32, tag="upw")

            # Engine observers: absorb the smalls-DMA sems once per engine so
            # later instructions carry only their single data wait.
            act_obs = persist.tile([1, 1], F32, tag="act_obs")
            nc.scalar.copy(out=act_obs, in_=smc[0:1, SMC_BS : SMC_BS + 1])

            # PE warm-up: hold tensor-engine p-state high before real work.
            # Dummies share the uu PSUM bank (unused until the uu matmuls).
            for i in range(7):
                nc.tensor.matmul(
                    upw[0:1, :448],
                    lhsT=warm[:, 0:1],
                    rhs=warm[:, :448],
                    start=True,
                    stop=True,
                )
            # PE observers for the two smalls DMAs (one wait each)
            nc.tensor.matmul(
                upw[0:1, 0:1],
                lhsT=smc[0:P, 0:1].bitcast(BF16)[:, 0:1],
                rhs=smc[0:P, 0:1].bitcast(BF16)[:, 0:1],
                start=True,
                stop=True,
            )
            nc.tensor.matmul(
                upw[0:1, 0:1],
                lhsT=smb[0:1, SMB_ONE : SMB_ONE + 1].bitcast(BF16)[:, 0:1],
                rhs=smb[0:1, SMB_ONE : SMB_ONE + 1].bitcast(BF16)[:, 0:1],
                start=True,
                stop=True,
            )

            # stream x in, alternating between the SP and ACT hardware DGE
            # queues so transfers overlap across the 16 DMA engines
            for j, (d0, dw) in enumerate(DCHUNKS):
                eng = nc.sync if j % 2 == 0 else nc.scalar
                eng.dma_start(
                    out=xh[:, :, d0 : d0 + dw],
                    in_=xh_in[:, d0 : d0 + dw].rearrange("(t p) n -> p t n", p=P),
                )

            # in-phase per compute chunk: matvec + relu; xsum accum per dma chunk
            di = 0
            dmap = {v: k for k, v in DFIRST.items()}
            for ci, (c0, w) in enumerate(CCHUNKS):
                if ci in dmap:
                    dd0, _ = DCHUNKS[dmap[ci]]
                    # PE observer: absorb this dma chunk's sem once
                    nc.tensor.matmul(
                        upw[0:1, 0:1],
                        lhsT=xh[:, 0, dd0 : dd0 + 1],
                        rhs=xh[:, 0, dd0 : dd0 + 1],
                        start=True,
                        stop=True,
                    )
                zp = ps_z.tile([1, CW], F32, tag="zp")
                # dep-free warm matmul carries the psum-bank WAR wait and
                # keeps the PE p-state up while waiting for the next chunk
                nc.tensor.matmul(
                    zp[:, :128],
                    lhsT=warm[:, 0:1],
                    rhs=warm[:, :128],
                    start=True,
                    stop=True,
                )
                for t in range(NT):
                    nc.tensor.matmul(
                        zp[:, :w],
                        lhsT=smcbf(t),
                        rhs=xh[:, t, c0 : c0 + w],
                        start=(t == 0),
                        stop=(t == NT - 1),
                    )
                nc.scalar.activation(
                    out=s_row[0:1, c0 : c0 + w],
                    in_=zp[0:1, :w],
                    func=mybir.ActivationFunctionType.Relu,
                    bias=smc[0:1, SMC_BS : SMC_BS + 1],
                    scale=1.0,
                )
                # xsum partials once the covering dma chunk is complete
                if ci in (0, 2, 4, 6):
                    d0, dw = DCHUNKS[di]
                    for t in range(NT):
                        nc.vector.tensor_scalar(
                            out=junk[:, t, di, :dw],
                            in0=xh[:, t, d0 : d0 + dw],
                            scalar1=1.0,
                            scalar2=0.0,
                            op0=mybir.AluOpType.mult,
                            op1=mybir.AluOpType.add,
                            accum_out=xsp[:, t, di : di + 1],
                        )
                    di += 1

            # s-broadcasts for the D/P output paths (depend only on relus, so
            # emitted before the uu matmuls: PE is in-order and can run them
            # while waiting for the last dma chunk).
            sb_psum = {}
            for ci, (c0, w) in enumerate(CCHUNKS):
                if "D" in PATHS[ci]:
                    sbp = ps_b.tile([P, CW], F32, tag="sbp")
                    nc.tensor.matmul(
                        sbp[:, :w],
                        lhsT=smbbf(0, 1, SMB_ONE, SMB_ONE + P // 2),
                        rhs=s_row[0:1, c0 : c0 + w],
                        start=True,
                        stop=True,
                    )
                    sb_psum[ci] = sbp
                if "P" in PATHS[ci]:
                    o5 = SB5_OFF[ci]
                    nc.gpsimd.partition_broadcast(
                        out_ap=sb5[:, o5 : o5 + w],
                        in_ap=s_row[0:1, c0 : c0 + w],
                    )

            # xsum -> uu (row and column forms)
            nc.vector.tensor_reduce(
                out=xsum[:, :, :],
                in_=xsp[:, :, :],
                op=mybir.AluOpType.add,
                axis=mybir.AxisListType.X,
            )
            nc.vector.tensor_copy(out=xsum_bf[:, :], in_=xsum[:, :, 0])

            one_bf = smbbf(0, 1, SMB_ONE, SMB_ONE + 1)[:, 0:1]
            up = upw[0:1, :C]
            nc.tensor.matmul(
                up[:, :],
                lhsT=one_bf,
                rhs=smbbf(0, 1, SMB_WBV, SMB_WBV + C // 2),
                start=True,
                stop=False,
                skip_group_check=True,
            )
            for t in range(NT):
                nc.tensor.matmul(
                    up[:, :],
                    lhsT=xsum_bf[:, t : t + 1],
                    rhs=smbbf(0, P, SMB_WVET + t * P, SMB_WVET + (t + 1) * P),
                    start=False,
                    stop=(t == NT - 1),
                    skip_group_check=True,
                )
            nc.scalar.copy(out=uu_row[:, :], in_=up[:, :])

            ucp = upw[:, 448:450]
            for m in range(NT):
                for tk in range(NT):
                    nc.tensor.matmul(
                        ucp[:, m : m + 1],
                        lhsT=smbbf(0, P, SMB_WVET + tk * P, SMB_WVET + (tk + 1) * P)[
                            :, m * P : (m + 1) * P
                        ],
                        rhs=xsum_bf[:, tk : tk + 1],
                        start=(tk == 0),
                        stop=False,
                        skip_group_check=True,
                    )
                nc.tensor.matmul(
                    ucp[:, m : m + 1],
                    lhsT=smbbf(0, 1, SMB_WBV + m * (C // 4), SMB_WBV + (m + 1) * (C // 4)),
                    rhs=one_bf,
                    start=False,
                    stop=True,
                    skip_group_check=True,
                )
            # gpsimd cannot read PSUM: stage the column-form uu in SBUF
            nc.vector.tensor_copy(out=uu_col[:, :], in_=ucp[:, :])

            # output phase
            for ci, (c0, w) in enumerate(CCHUNKS):
                pt = PATHS[ci]
                for t in range(NT):
                    path = pt[t]
                    if path == "D":
                        nc.vector.scalar_tensor_tensor(
                            out=obf[:, t, c0 : c0 + w],
                            in0=sb_psum[ci][:, :w],
                            scalar=ucp[:, t : t + 1],
                            in1=xh[:, t, c0 : c0 + w],
                            op0=mybir.AluOpType.mult,
                            op1=mybir.AluOpType.add,
                        )
                    elif path == "P":
                        o5 = SB5_OFF[ci]
                        nc.gpsimd.scalar_tensor_tensor(
                            out=obf[:, t, c0 : c0 + w],
                            in0=sb5[:, o5 : o5 + w],
                            scalar=uu_col[:, t : t + 1],
                            in1=xh[:, t, c0 : c0 + w],
                            op0=mybir.AluOpType.mult,
                            op1=mybir.AluOpType.add,
                        )
                    else:  # ACT path
                        opst = ps_o.tile([P, CW], F32, tag="opst")
                        nc.tensor.matmul(
                            opst[:, :w],
                            lhsT=uu_row[0:1, t * P : (t + 1) * P],
                            rhs=s_row[0:1, c0 : c0 + w],
                            start=True,
                            stop=False,
                        )
                        nc.tensor.matmul(
                            opst[:, :w],
                            lhsT=smbbf(0, P, SMB_I, SMB_I + P // 2),
                            rhs=xh[:, t, c0 : c0 + w],
                            start=False,
                            stop=True,
                        )
                        nc.scalar.copy(
                            out=obf[:, t, c0 : c0 + w], in_=opst[:, :w]
                        )
                nc.sync.dma_start(
                    out=out[:, c0 : c0 + w].rearrange("(t p) n -> p t n", p=P),
                    in_=obf[:, :, c0 : c0 + w],
                )
    _split_multi_waits(nc)
    return nc


def _pack_smalls(Wq, bq, Wk, bk, Wv, bv, Wcat, Wexp, bexp):
    import ml_dtypes

    f32 = np.float32
    wq_c, wk_c = Wcat[0, :E], Wcat[0, E:]
    wS = (Wq.T @ wq_c + Wk.T @ wk_c).astype(f32)  # [C]
    bS = f32(wq_c @ bq + wk_c @ bk) - f32(wS @ bexp)
    Wve = (Wexp @ Wv).astype(f32)  # [C, C]
    wvet = (Wve.T / f32(N)).astype(f32)  # [k, m]
    wexpbv = (Wexp @ bv - Wve @ bexp).astype(f32)

    def bf(x):
        return np.asarray(x, f32).astype(ml_dtypes.bfloat16).view(np.uint16)

    smc = np.zeros((P, SMC_F), f32)
    u16c = smc.view(np.uint16).reshape(P, SMC_F, 2)
    for t in range(NT):
        u16c[:, SMC_WS + t, 0] = bf(wS[t * P : (t + 1) * P])
    smc[0, SMC_BS] = bS

    smb = np.zeros((P, SMB_F), f32)
    u16b = smb.view(np.uint16).reshape(P, SMB_F, 2)
    for t in range(NT):
        u16b[:, SMB_WVET + t * P : SMB_WVET + (t + 1) * P, :] = bf(
            wvet[t * P : (t + 1) * P, :]
        ).reshape(P, P, 2)
    u16b[0, SMB_WBV : SMB_WBV + C // 2, :] = bf(wexpbv).reshape(C // 2, 2)
    ident = np.eye(P, dtype=f32)
    u16b[:, SMB_I : SMB_I + P // 2, :] = bf(ident).reshape(P, P // 2, 2)
    u16b[0, SMB_ONE : SMB_ONE + P // 2, :] = bf(np.ones(P, f32)).reshape(P // 2, 2)
    return smc, smb


def kernel(x, Wq, bq, Wk, bk, Wv, bv, Wcat, Wexp, bexp):
    global LAST_RESULTS
    import ml_dtypes

    f32 = np.float32
    x = np.asarray(x, f32)
    args = [np.asarray(a, f32) for a in (Wq, bq, Wk, bk, Wv, bv, Wcat, Wexp, bexp)]
    smc, smb = _pack_smalls(*args)
    bexp = args[-1]

    if "prog" not in _prog_cache:
        _prog_cache["prog"] = _build()
    nc = _prog_cache["prog"]

    xh = (x.reshape(B, C, N) + bexp[None, :, None]).astype(ml_dtypes.bfloat16)
    in_maps = [
        {"xh": np.ascontiguousarray(xh[b]), "smc": smc, "smb": smb}
        for b in range(B)
    ]

    LAST_RESULTS = run_bass_kernel_spmd(nc, in_maps, core_ids=list(range(B)))
    out = np.stack(
        [LAST_RESULTS.results[b]["out"] for b in range(B)], axis=0
    ).astype(f32)
    return out.reshape(B, C, H, W)


if __name__ == "__main__":
    rng = np.random.default_rng(0)
    s = 0.02
    f32 = np.float32
    args = dict(
        x=rng.standard_normal((B, C, H, W)).astype(f32),
        Wq=(rng.standard_normal((E, C)) * s).astype(f32),
        bq=(rng.standard_normal((E,)) * s).astype(f32),
        Wk=(rng.standard_normal((E, C)) * s).astype(f32),
        bk=(rng.standard_normal((E,)) * s).astype(f32),
        Wv=(rng.standard_normal((E, C)) * s).astype(f32),
        bv=(rng.standard_normal((E,)) * s).astype(f32),
        Wcat=(rng.standard_normal((1, 2 * E)) * s).astype(f32),
        Wexp=(rng.standard_normal((C, E)) * s).astype(f32),
        bexp=(rng.standard_normal((C,)) * s).astype(f32),
    )
    o = kernel(**args)
    print(o.shape, o.dtype)

